# revision 1
# baseline (speedup 1.0000x reference)
"""GatedSlotAttention2 Trainium2 Bass kernel.

Sharding: 2 heads per core x 8 cores (H=16). Each core computes its two
heads' full pipeline (projections -> short conv -> chunked gated-slot scan
-> RMSNorm-gate -> partial Wo matmul); host sums the 8 partial outputs.

Scan algorithm: chunk-parallel reformulation of the per-step recurrence
with chunk size C=64 (validated vs the sequential reference to ~6e-7 in
f32; bf16 projections give ~4e-3).
"""
import numpy as np
import ml_dtypes

import concourse.bass as bass
import concourse.bacc as bacc_mod
import concourse.mybir as mybir
import concourse.tile as tile
from concourse.bass_utils import run_bass_kernel_spmd

F32 = mybir.dt.float32
BF16 = mybir.dt.bfloat16
AF = mybir.ActivationFunctionType
ALU = mybir.AluOpType
MS = bass.MemorySpace

B, T, HID = 1, 1024, 2048
H, DK, DV, M, KW = 16, 128, 128, 128, 4
SCALE = DK ** -0.5
EPS = 1e-5
C = 64            # chunk length
NCH = T // C      # 16 chunks
NKT = HID // 128  # 16 contraction tiles
HL = 2            # heads per core

_CACHE = {}


def _build_nc():
    nc = bacc_mod.Bacc("TRN2")

    # ---------------- DRAM I/O ----------------
    d_xt = nc.dram_tensor("xt", [HID, T], BF16, kind="ExternalInput")        # X^T
    d_wq = nc.dram_tensor("wq", [HID, HL * DK], BF16, kind="ExternalInput")
    d_wk = nc.dram_tensor("wk", [HID, HL * DK], BF16, kind="ExternalInput")
    d_wv = nc.dram_tensor("wv", [HID, HL * DV], BF16, kind="ExternalInput")
    d_ww = nc.dram_tensor("ww", [HID, HL * M], BF16, kind="ExternalInput")
    d_wf1 = nc.dram_tensor("wf1", [HID, DV], BF16, kind="ExternalInput")
    d_wg1 = nc.dram_tensor("wg1", [HID, DV], BF16, kind="ExternalInput")
    d_wb = nc.dram_tensor("wb", [HID, HL], BF16, kind="ExternalInput")
    d_wf2 = nc.dram_tensor("wf2", [DV, HL * M], F32, kind="ExternalInput")
    d_wg2 = nc.dram_tensor("wg2", [DV, HL * DV], F32, kind="ExternalInput")
    d_bg2 = nc.dram_tensor("bg2", [1, HL * DV], F32, kind="ExternalInput")
    d_wo = nc.dram_tensor("wo", [HL * DV, HID], BF16, kind="ExternalInput")  # norm_w folded
    d_cq = nc.dram_tensor("cq", [128, HL, KW], F32, kind="ExternalInput")
    d_ck = nc.dram_tensor("ck", [128, HL, KW], F32, kind="ExternalInput")
    d_cv = nc.dram_tensor("cv", [128, HL, KW], F32, kind="ExternalInput")
    # constants
    d_trineg = nc.dram_tensor("trineg", [C, C], F32, kind="ExternalInput")       # -1 if j<=i
    d_trirev = nc.dram_tensor("trirev", [C, C], F32, kind="ExternalInput")       # -1 if j>i
    d_negc31 = nc.dram_tensor("negc31", [C, C], F32, kind="ExternalInput")       # -1 if j<=31
    d_maskS = nc.dram_tensor("masks", [C, C], F32, kind="ExternalInput")         # SCALE if j<=i
    d_maskJ = nc.dram_tensor("maskj", [C, C], mybir.dt.uint8, kind="ExternalInput")         # 1 if j<=i
    d_negones = nc.dram_tensor("negones", [C, 128], F32, kind="ExternalInput")   # all -1
    d_ident = nc.dram_tensor("ident", [128, 128], F32, kind="ExternalInput")
    d_ones1 = nc.dram_tensor("ones1", [1, C], F32, kind="ExternalInput")         # ones row

    d_out = nc.dram_tensor("out", [T, HID], F32, kind="ExternalOutput")

    with tile.TileContext(nc) as tc:
        with (
            tc.tile_pool(name="persist", bufs=1) as pp,
            tc.tile_pool(name="wpool", bufs=2) as wp,
            tc.tile_pool(name="convT", bufs=2) as cvp,
            tc.tile_pool(name="xpad", bufs=2) as xpp,
            tc.tile_pool(name="scr", bufs=2) as scr,
            tc.tile_pool(name="ps_proj", bufs=2, space=MS.PSUM) as ps_proj,
            tc.tile_pool(name="ps_scan", bufs=4, space=MS.PSUM) as ps_scan,
            tc.tile_pool(name="ps_out", bufs=2, space=MS.PSUM) as ps_out,
        ):
            # ---------- constants to SBUF ----------
            def load_const(dram, shape, dtype=F32):
                t = pp.tile(shape, dtype, tag=dram.name + "_sb")
                nc.sync.dma_start(t[:], dram[:])
                return t

            c_trineg = load_const(d_trineg, [C, C])
            c_trirev = load_const(d_trirev, [C, C])
            c_negc31 = load_const(d_negc31, [C, C])
            c_maskS = load_const(d_maskS, [C, C])
            c_maskJ = load_const(d_maskJ, [C, C], mybir.dt.uint8)
            c_negones = load_const(d_negones, [C, 128])
            c_ident = load_const(d_ident, [128, 128])
            c_ones1 = load_const(d_ones1, [1, C])
            c_wf2 = load_const(d_wf2, [DV, HL * M])
            c_wg2 = load_const(d_wg2, [DV, HL * DV])
            c_bg2 = load_const(d_bg2, [1, HL * DV])
            c_cq = load_const(d_cq, [128, HL, KW])
            c_ck = load_const(d_ck, [128, HL, KW])
            c_cv = load_const(d_cv, [128, HL, KW])
            c_eps6 = pp.tile([C, 1], F32, tag="c_eps6")
            nc.vector.memset(c_eps6[:], 1e-6)
            c_eps5 = pp.tile([C, 1], F32, tag="c_eps5")
            nc.vector.memset(c_eps5[:], EPS)

            # ---------- X^T stream tiles + big weights ----------
            xt_sb = pp.tile([128, NKT, T], BF16, tag="xt_sb")
            xtr = d_xt.rearrange("(k p) t -> k p t", p=128)
            for kt in range(NKT):
                nc.sync.dma_start(xt_sb[:, kt, :], xtr[kt])

            wo_sb = pp.tile([128, HL, HID], BF16, tag="wo_sb")
            wor = d_wo.rearrange("(h p) o -> h p o", p=128)
            for h in range(HL):
                nc.sync.dma_start(wo_sb[:, h, :], wor[h])

            # ---------- projections + conv + silu ----------
            # conv outputs, [channel, t] layout; q/k persist, v/w rotate
            qT = pp.tile([128, HL, T], F32, tag="qT")
            kT = pp.tile([128, HL, T], F32, tag="kT")

            def project_convT(d_w, c_cw, out_tile, name):
                """out[ct][c,t] = silu(conv1d(W[:,c].T @ X^T, cw)) per c-tile."""
                w_sb = wp.tile([128, NKT, HL * 128], BF16, tag="w_load")
                wr = d_w.rearrange("(k p) c -> k p c", p=128)
                for kt in range(NKT):
                    nc.sync.dma_start(w_sb[:, kt, :], wr[kt])
                for ct in range(HL):
                    acc = [None, None]
                    for tt in range(2):
                        ps = ps_proj.tile([128, 512], F32, tag="pp")
                        for kt in range(NKT):
                            nc.tensor.matmul(
                                ps[:],
                                w_sb[:, kt, ct * 128:(ct + 1) * 128],
                                xt_sb[:, kt, tt * 512:(tt + 1) * 512],
                                start=(kt == 0), stop=(kt == NKT - 1),
                            )
                        acc[tt] = ps
                    xpad = xpp.tile([128, T + KW - 1], F32, tag="xpad")
                    nc.vector.memset(xpad[:, 0:KW - 1], 0.0)
                    for tt in range(2):
                        nc.vector.tensor_copy(
                            xpad[:, KW - 1 + tt * 512: KW - 1 + (tt + 1) * 512],
                            acc[tt][:])
                    cacc = xpp.tile([128, T], F32, tag="convacc")
                    nc.vector.tensor_scalar_mul(
                        cacc[:], xpad[:, 0:T], c_cw[:, ct, 0:1])
                    for i in range(1, KW):
                        nc.vector.scalar_tensor_tensor(
                            cacc[:], xpad[:, i:i + T], c_cw[:, ct, i:i + 1],
                            cacc[:], op0=ALU.mult, op1=ALU.add)
                    se = xpp.tile([128, T], F32, tag="se")
                    nc.scalar.activation(se[:], cacc[:], AF.Exp, scale=-1.0)
                    nc.vector.tensor_scalar_add(se[:], se[:], 1.0)
                    nc.vector.reciprocal(se[:], se[:])
                    nc.vector.tensor_mul(out_tile[:, ct, :], cacc[:], se[:])

            project_convT(d_wq, c_cq, qT, "q")
            project_convT(d_wk, c_ck, kT, "k")
            vT = cvp.tile([128, HL, T], F32, tag="convT")
            project_convT(d_wv, c_cv, vT, "v")
            wT = cvp.tile([128, HL, T], F32, tag="convT")
            project_convT(d_ww, c_cv, wT, "w")

            # ---------- gate-path projections: F1T, G1T, betaT ----------
            def proj128T(d_w, tag):
                out = pp.tile([128, T], F32, tag=tag)
                w_sb = wp.tile([128, NKT, 128], BF16, tag="w_load")
                wr = d_w.rearrange("(k p) c -> k p c", p=128)
                for kt in range(NKT):
                    nc.sync.dma_start(w_sb[:, kt, :], wr[kt])
                for tt in range(2):
                    ps = ps_proj.tile([128, 512], F32, tag="pp")
                    for kt in range(NKT):
                        nc.tensor.matmul(
                            ps[:], w_sb[:, kt, :],
                            xt_sb[:, kt, tt * 512:(tt + 1) * 512],
                            start=(kt == 0), stop=(kt == NKT - 1))
                    nc.scalar.copy(out[:, tt * 512:(tt + 1) * 512], ps[:])
                return out

            f1T = proj128T(d_wf1, "f1T")
            g1T = proj128T(d_wg1, "g1T")

            betaT = pp.tile([HL, T], F32, tag="betaT")
            wb_sb = wp.tile([128, NKT, HL], BF16, tag="wb_load")
            wbr = d_wb.rearrange("(k p) c -> k p c", p=128)
            for kt in range(NKT):
                nc.sync.dma_start(wb_sb[:, kt, :], wbr[kt])
            for tt in range(2):
                ps = ps_proj.tile([HL, 512], F32, tag="pp")
                for kt in range(NKT):
                    nc.tensor.matmul(
                        ps[:], wb_sb[:, kt, :],
                        xt_sb[:, kt, tt * 512:(tt + 1) * 512],
                        start=(kt == 0), stop=(kt == NKT - 1))
                bsl = betaT[:, tt * 512:(tt + 1) * 512]
                nc.scalar.activation(bsl, ps[:], AF.Exp, scale=-1.0)
                nc.vector.tensor_scalar_add(bsl, bsl, 1.0)
                nc.vector.reciprocal(bsl, bsl)

            # ---------- states ----------
            Sk = [pp.tile([DK, M], F32, name=f"Sk{h}", tag=f"Sk{h}") for h in range(HL)]
            Sv = [pp.tile([M, DV], F32, name=f"Sv{h}", tag=f"Sv{h}") for h in range(HL)]
            for h in range(HL):
                nc.vector.memset(Sk[h][:], 0.0)
                nc.vector.memset(Sv[h][:], 0.0)

            oT = [pp.tile([DV, NCH, C], BF16, name=f"oT{h}", tag=f"oT{h}") for h in range(HL)]

            # ---------- chunked scan ----------
            for n in range(NCH):
                t0 = n * C
                # shared across the two heads: gpos/gate/beta for this chunk
                gps = ps_scan.tile([C, HL * M], F32, tag="ps")
                nc.tensor.matmul(gps[:], f1T[:, t0:t0 + C], c_wf2[:],
                                 start=True, stop=True)
                gpos = scr.tile([C, HL * M], F32, tag="gpos")
                nc.scalar.activation(gpos[:], gps[:], AF.Exp, scale=-1.0)
                nc.scalar.activation(gpos[:], gpos[:], AF.Ln, bias=1.0)

                gt_ps = ps_scan.tile([C, HL * DV], F32, tag="ps")
                nc.tensor.matmul(gt_ps[:], g1T[:, t0:t0 + C], c_wg2[:],
                                 start=True, stop=False)
                nc.tensor.matmul(gt_ps[:], c_ones1[:], c_bg2[:],
                                 start=False, stop=True)
                gate = scr.tile([C, HL * DV], F32, tag="gate")
                nc.scalar.activation(gate[:], gt_ps[:], AF.Exp, scale=-1.0)
                nc.vector.tensor_scalar_add(gate[:], gate[:], 1.0)
                nc.vector.reciprocal(gate[:], gate[:])

                bt_ps = ps_scan.tile([C, HL], F32, tag="ps")
                nc.tensor.transpose(bt_ps[:], betaT[:, t0:t0 + C],
                                    c_ident[0:HL, 0:HL])
                beta = scr.tile([C, HL], F32, tag="beta")
                nc.scalar.copy(beta[:], bt_ps[:])

                for h in range(HL):
                    hs = slice(h * 128, (h + 1) * 128)
                    # --- per-chunk transposes: K, V, W ---
                    kps = ps_scan.tile([C, 128], F32, tag="ps")
                    nc.tensor.transpose(kps[:], kT[:, h, t0:t0 + C], c_ident[:])
                    Kc = scr.tile([C, 128], F32, tag="Kc")
                    nc.scalar.copy(Kc[:], kps[:])

                    vps = ps_scan.tile([C, 128], F32, tag="ps")
                    nc.tensor.transpose(vps[:], vT[:, h, t0:t0 + C], c_ident[:])
                    Vc = scr.tile([C, 128], F32, tag="Vc")
                    nc.scalar.copy(Vc[:], vps[:])

                    wps = ps_scan.tile([C, 128], F32, tag="ps")
                    nc.tensor.transpose(wps[:], wT[:, h, t0:t0 + C], c_ident[:])
                    # l2norm + beta scaling -> bw
                    w2 = scr.tile([C, 128], F32, tag="w2")
                    ss = scr.tile([C, 1], F32, tag="ss")
                    nc.scalar.activation(w2[:], wps[:], AF.Square, accum_out=ss[:])
                    sd = scr.tile([C, 1], F32, tag="sd")
                    nc.scalar.activation(sd[:], ss[:], AF.Ln, bias=c_eps6[:])
                    rs = scr.tile([C, 1], F32, tag="rs")
                    nc.scalar.activation(rs[:], sd[:], AF.Exp, scale=-0.5)
                    rsb = scr.tile([C, 1], F32, tag="rsb")
                    nc.vector.tensor_mul(rsb[:], rs[:], beta[:, h:h + 1])
                    bw = scr.tile([C, 128], F32, tag="bw")
                    nc.vector.tensor_scalar_mul(bw[:], wps[:], rsb[:])

                    # --- gate cumsums (via triangular matmuls) ---
                    gsl = gpos[:, hs]
                    gc_ps = ps_scan.tile([C, M], F32, tag="ps")
                    nc.tensor.matmul(gc_ps[:], c_trineg[:], gsl,
                                     start=True, stop=True)
                    Gc = scr.tile([C, M], F32, tag="Gc")
                    nc.scalar.copy(Gc[:], gc_ps[:])
                    grev_ps = ps_scan.tile([C, M], F32, tag="ps")
                    nc.tensor.matmul(grev_ps[:], c_trirev[:], gsl,
                                     start=True, stop=True)
                    b1_ps = ps_scan.tile([C, M], F32, tag="ps")
                    nc.tensor.matmul(b1_ps[:], c_negc31[:], gsl,
                                     start=True, stop=True)
                    Gcp = scr.tile([C, M], F32, tag="Gcp")
                    nc.vector.tensor_sub(Gcp[:], Gc[:], b1_ps[:])
                    Lam = scr.tile([C, M], F32, tag="Lam")
                    nc.scalar.activation(Lam[:], Gc[:], AF.Exp)
                    Epos = scr.tile([C, M], F32, tag="Epos")
                    nc.scalar.activation(Epos[:], Gcp[:], AF.Exp)
                    Enege = scr.tile([C, M], F32, tag="Enege")
                    nc.scalar.activation(Enege[:], Gcp[:], AF.Exp, scale=-1.0)
                    Eneg = scr.tile([C, M], F32, tag="Eneg")
                    nc.vector.tensor_mul(Eneg[:], Enege[:], bw[:])
                    Ereve = scr.tile([C, M], F32, tag="Ereve")
                    nc.scalar.activation(Ereve[:], grev_ps[:], AF.Exp)
                    Kdec = scr.tile([C, M], F32, tag="Kdec")
                    nc.vector.tensor_mul(Kdec[:], Ereve[:], bw[:])

                    # chunk-end decay broadcasts
                    lcb_ps = ps_scan.tile([128, M], F32, tag="ps")
                    nc.tensor.matmul(lcb_ps[:], c_negones[:], gsl,
                                     start=True, stop=True)
                    LamCb = scr.tile([128, M], F32, tag="LamCb")
                    nc.scalar.activation(LamCb[:], lcb_ps[:], AF.Exp)
                    lcc_ps = ps_scan.tile([M, 1], F32, tag="ps")
                    nc.tensor.matmul(lcc_ps[:], gsl, c_negones[:, 0:1],
                                     start=True, stop=True)
                    LamCc = scr.tile([M, 1], F32, tag="LamCc")
                    nc.scalar.activation(LamCc[:], lcc_ps[:], AF.Exp)

                    # --- pass A: scores + softmax ---
                    pt_ps = ps_scan.tile([C, C], F32, tag="ps")
                    nc.tensor.matmul(pt_ps[:], kT[:, h, t0:t0 + C],
                                     qT[:, h, t0:t0 + C], start=True, stop=True)
                    Ptm = scr.tile([C, C], F32, tag="Ptm")
                    nc.vector.tensor_mul(Ptm[:], pt_ps[:], c_maskS[:])
                    intra_ps = ps_scan.tile([C, M], F32, tag="ps")
                    nc.tensor.matmul(intra_ps[:], Ptm[:], Eneg[:],
                                     start=True, stop=True)
                    qs_ps = ps_scan.tile([C, M], F32, tag="ps")
                    nc.tensor.matmul(qs_ps[:], qT[:, h, t0:t0 + C], Sk[h][:],
                                     start=True, stop=True)
                    s1 = scr.tile([C, M], F32, tag="s1")
                    nc.vector.scalar_tensor_tensor(
                        s1[:], qs_ps[:], SCALE, Lam[:],
                        op0=ALU.mult, op1=ALU.mult)
                    s2 = scr.tile([C, M], F32, tag="s2")
                    nc.vector.tensor_mul(s2[:], intra_ps[:], Epos[:])
                    sS = scr.tile([C, M], F32, tag="sS")
                    nc.vector.tensor_add(sS[:], s1[:], s2[:])
                    mx = scr.tile([C, 1], F32, tag="mx")
                    nc.vector.tensor_reduce(mx[:], sS[:], mybir.AxisListType.X,
                                            ALU.max)
                    nmx = scr.tile([C, 1], F32, tag="nmx")
                    nc.vector.tensor_scalar_mul(nmx[:], mx[:], -1.0)
                    pexp = scr.tile([C, M], F32, tag="pexp")
                    den = scr.tile([C, 1], F32, tag="den")
                    nc.scalar.activation(pexp[:], sS[:], AF.Exp, bias=nmx[:],
                                         accum_out=den[:])
                    rec = scr.tile([C, 1], F32, tag="rec")
                    nc.vector.reciprocal(rec[:], den[:])
                    aL = scr.tile([C, M], F32, tag="aL")
                    nc.vector.scalar_tensor_tensor(
                        aL[:], pexp[:], rec[:], Lam[:],
                        op0=ALU.mult, op1=ALU.mult)
                    aE = scr.tile([C, M], F32, tag="aE")
                    nc.vector.scalar_tensor_tensor(
                        aE[:], pexp[:], rec[:], Epos[:],
                        op0=ALU.mult, op1=ALU.mult)

                    # --- pass B: output ---
                    alt_ps = ps_scan.tile([M, C], F32, tag="ps")
                    nc.tensor.transpose(alt_ps[:], aL[:], c_ident[0:C, 0:C])
                    aLT = scr.tile([M, C], F32, tag="aLT")
                    nc.scalar.copy(aLT[:], alt_ps[:])
                    aet_ps = ps_scan.tile([M, C], F32, tag="ps")
                    nc.tensor.transpose(aet_ps[:], aE[:], c_ident[0:C, 0:C])
                    aET = scr.tile([M, C], F32, tag="aET")
                    nc.scalar.copy(aET[:], aet_ps[:])
                    ent_ps = ps_scan.tile([M, C], F32, tag="ps")
                    nc.tensor.transpose(ent_ps[:], Eneg[:], c_ident[0:C, 0:C])
                    EnegT = scr.tile([M, C], F32, tag="EnegT")
                    nc.scalar.copy(EnegT[:], ent_ps[:])

                    rt_ps = ps_scan.tile([C, C], F32, tag="ps")
                    nc.tensor.matmul(rt_ps[:], EnegT[:], aET[:],
                                     start=True, stop=True)
                    Rmt = scr.tile([C, C], F32, tag="Rmt")
                    nc.vector.memset(Rmt[:], 0.0)
                    nc.vector.copy_predicated(Rmt[:], c_maskJ[:], rt_ps[:])

                    o_ps = ps_scan.tile([C, DV], F32, tag="ps")
                    nc.tensor.matmul(o_ps[:], aLT[:], Sv[h][:],
                                     start=True, stop=False)
                    nc.tensor.matmul(o_ps[:], Rmt[:], Vc[:],
                                     start=False, stop=True)

                    # --- state updates ---
                    skk_ps = ps_scan.tile([DK, M], F32, tag="ps")
                    nc.tensor.matmul(skk_ps[:], Kc[:], Kdec[:],
                                     start=True, stop=True)
                    skt = scr.tile([DK, M], F32, tag="skt")
                    nc.vector.tensor_mul(skt[:], Sk[h][:], LamCb[:])
                    nc.vector.tensor_add(Sk[h][:], skt[:], skk_ps[:])
                    svk_ps = ps_scan.tile([M, DV], F32, tag="ps")
                    nc.tensor.matmul(svk_ps[:], Kdec[:], Vc[:],
                                     start=True, stop=True)
                    svt = scr.tile([M, DV], F32, tag="svt")
                    nc.vector.tensor_scalar_mul(svt[:], Sv[h][:], LamCc[:])
                    nc.vector.tensor_add(Sv[h][:], svt[:], svk_ps[:])

                    # --- epilogue: RMSNorm * sigmoid(gate), transpose ---
                    o2 = scr.tile([C, DV], F32, tag="o2")
                    oss = scr.tile([C, 1], F32, tag="oss")
                    nc.scalar.activation(o2[:], o_ps[:], AF.Square,
                                         accum_out=oss[:])
                    orm = scr.tile([C, 1], F32, tag="orm")
                    nc.scalar.activation(orm[:], oss[:], AF.Ln,
                                         scale=1.0 / DV, bias=c_eps5[:])
                    orr = scr.tile([C, 1], F32, tag="orr")
                    nc.scalar.activation(orr[:], orm[:], AF.Exp, scale=-0.5)
                    o1 = scr.tile([C, DV], F32, tag="o1")
                    nc.vector.tensor_mul(o1[:], o_ps[:], gate[:, hs])
                    of = scr.tile([C, DV], F32, tag="of")
                    nc.vector.tensor_scalar_mul(of[:], o1[:], orr[:])
                    ot_ps = ps_scan.tile([DV, C], F32, tag="ps")
                    nc.tensor.transpose(ot_ps[:], of[:], c_ident[0:C, 0:C])
                    nc.scalar.copy(oT[h][:, n, :], ot_ps[:])

            # ---------- output projection (partial; host sums cores) ----------
            for tt in range(8):
                for cl in range(4):
                    ps = ps_out.tile([128, 512], F32, tag="po")
                    for h in range(HL):
                        nc.tensor.matmul(
                            ps[:],
                            oT[h][:, 2 * tt:2 * tt + 2, :],
                            wo_sb[:, h, cl * 512:(cl + 1) * 512],
                            start=(h == 0), stop=(h == HL - 1))
                    osb = scr.tile([128, 512], F32, tag="outsb", bufs=3)
                    nc.scalar.copy(osb[:], ps[:])
                    nc.sync.dma_start(
                        d_out[tt * 128:(tt + 1) * 128, cl * 512:(cl + 1) * 512],
                        osb[:])
    nc.compile()
    return nc


def _host_inputs(inputs):
    """Build the 8 per-core input maps from the full-problem inputs."""
    f32 = np.float32
    bf16 = ml_dtypes.bfloat16
    X = np.ascontiguousarray(np.asarray(inputs["hidden_states"], f32)[0])  # [T, HID]
    XT = np.ascontiguousarray(X.T).astype(bf16)

    tri_neg = np.triu(np.full((C, C), -1.0, f32))          # [j,i] -1 if j<=i
    tri_rev = np.tril(np.full((C, C), -1.0, f32), -1)      # -1 if j>i
    negc31 = np.zeros((C, C), f32); negc31[:32, :] = -1.0  # -1 if j<=31
    maskS = np.triu(np.full((C, C), SCALE, f32))
    maskJ = np.triu(np.ones((C, C), f32))
    negones = np.full((C, 128), -1.0, f32)
    ident = np.eye(128, dtype=f32)
    ones1 = np.ones((1, C), f32)

    Wo_full = np.asarray(inputs["Wo"], f32) * np.tile(
        np.asarray(inputs["norm_w"], f32), H)[:, None]

    in_maps = []
    for c in range(8):
        hsl = slice(c * HL * 128, (c + 1) * HL * 128)
        bsl = slice(c * HL, (c + 1) * HL)
        m = {
            "xt": XT,
            "wq": np.asarray(inputs["Wq"], f32)[:, hsl].astype(bf16),
            "wk": np.asarray(inputs["Wk"], f32)[:, hsl].astype(bf16),
            "wv": np.asarray(inputs["Wv"], f32)[:, hsl].astype(bf16),
            "ww": np.asarray(inputs["Ww"], f32)[:, hsl].astype(bf16),
            "wf1": np.asarray(inputs["Wf1"], f32).astype(bf16),
            "wg1": np.asarray(inputs["Wg1"], f32).astype(bf16),
            "wb": np.asarray(inputs["Wb"], f32)[:, bsl].astype(bf16),
            "wf2": np.ascontiguousarray(np.asarray(inputs["Wf2"], f32)[:, hsl]),
            "wg2": np.ascontiguousarray(np.asarray(inputs["Wg2"], f32)[:, hsl]),
            "bg2": np.ascontiguousarray(
                np.asarray(inputs["bg2"], f32)[None, hsl]),
            "wo": np.ascontiguousarray(Wo_full[hsl]).astype(bf16),
            "cq": np.ascontiguousarray(
                np.asarray(inputs["cq"], f32)[hsl].reshape(HL, 128, KW)
                .transpose(1, 0, 2)),
            "ck": np.ascontiguousarray(
                np.asarray(inputs["ck"], f32)[hsl].reshape(HL, 128, KW)
                .transpose(1, 0, 2)),
            "cv": np.ascontiguousarray(
                np.asarray(inputs["cv"], f32)[hsl].reshape(HL, 128, KW)
                .transpose(1, 0, 2)),
            "trineg": tri_neg, "trirev": tri_rev, "negc31": negc31,
            "masks": maskS, "maskj": maskJ.astype(np.uint8), "negones": negones,
            "ident": ident, "ones1": ones1,
        }
        in_maps.append(m)
    return in_maps


def kernel(_trace=False, **inputs):
    if "nc" not in _CACHE:
        _CACHE["nc"] = _build_nc()
    nc = _CACHE["nc"]
    in_maps = _host_inputs(inputs)
    res = run_bass_kernel_spmd(nc, in_maps, core_ids=list(range(8)),
                               trace=_trace)
    _CACHE["last_result"] = res
    out = np.zeros((T, HID), np.float32)
    for r in res.results:
        out += r["out"]
    return out.reshape(B, T, HID)



# revision 9
# speedup vs baseline: 2.3489x; 2.3489x over previous
"""GatedSlotAttention2 Trainium2 Bass kernel (v2).

Sharding: 2 heads per core x 8 cores (H=16); host sums the 8 partial
Wo outputs. Chunked scan with C=128, all heavy matmul operands in bf16,
state-independent work hoisted out of the sequential loop, single
act-table discipline (Exp/Tanh/Square/Copy + two batched Ln regions),
softmax denominator folded into the RMSNorm eps term.
"""
import numpy as np
import ml_dtypes

import concourse.bass as bass
import concourse.bacc as bacc_mod
import concourse.mybir as mybir
import concourse.tile as tile
from concourse.bass_utils import run_bass_kernel_spmd

F32 = mybir.dt.float32
BF16 = mybir.dt.bfloat16
FP16 = mybir.dt.float16
AF = mybir.ActivationFunctionType
ALU = mybir.AluOpType
MS = bass.MemorySpace

B, T, HID = 1, 1024, 2048
H, DK, DV, M, KW = 16, 128, 128, 128, 4
SCALE = DK ** -0.5
EPS = 1e-5
C = 128           # chunk length
HC = C // 2
NCH = T // C      # 8 chunks
NKT = HID // 128  # 16 contraction tiles
HL = 2            # heads per core
NCT = 10          # 128-wide projection column tiles in wbig

_CACHE = {}


def _build_nc():
    nc = bacc_mod.Bacc("TRN2")

    # ---------------- DRAM I/O ----------------
    d_xt = nc.dram_tensor("xt", [HID, T], BF16, kind="ExternalInput")
    d_wbig = nc.dram_tensor("wbig", [HID, NCT * 128], BF16, kind="ExternalInput")
    d_wb = nc.dram_tensor("wb", [HID, HL], BF16, kind="ExternalInput")
    d_wf2 = nc.dram_tensor("wf2", [DV, HL * M], BF16, kind="ExternalInput")
    d_wg2 = nc.dram_tensor("wg2", [DV, HL * DV], BF16, kind="ExternalInput")
    d_bg2 = nc.dram_tensor("bg2", [1, HL * DV], BF16, kind="ExternalInput")
    d_wo = nc.dram_tensor("wo", [HL * DV, HID], BF16, kind="ExternalInput")
    d_convw = nc.dram_tensor("convw", [128, 8, KW], F32, kind="ExternalInput")
    d_mcum = nc.dram_tensor("mcum", [C, C], F32, kind="ExternalInput")
    d_mcen = nc.dram_tensor("mcen", [C, C], F32, kind="ExternalInput")
    d_mrev = nc.dram_tensor("mrev", [C, C], F32, kind="ExternalInput")
    d_negones = nc.dram_tensor("negones", [C, 128], F32, kind="ExternalInput")
    d_negcol = nc.dram_tensor("negcol", [C, 1], F32, kind="ExternalInput")
    d_trimask = nc.dram_tensor("trimask", [C, C], BF16, kind="ExternalInput")
    d_identb = nc.dram_tensor("identb", [128, 128], BF16, kind="ExternalInput")
    d_ones1 = nc.dram_tensor("ones1", [1, C], BF16, kind="ExternalInput")
    d_out = nc.dram_tensor("out", [T, HID], FP16, kind="ExternalOutput")

    with tile.TileContext(nc) as tc:
        with (
            tc.tile_pool(name="persist", bufs=1) as pp,
            tc.tile_pool(name="scr", bufs=3) as scr,
            tc.tile_pool(name="psA", bufs=2, space=MS.PSUM) as psA,
            tc.tile_pool(name="psB", bufs=2, space=MS.PSUM) as psB,
            tc.tile_pool(name="psC", bufs=4, space=MS.PSUM) as psC,
        ):
            # ---------- constants ----------
            def load_const(dram, shape, dtype=F32):
                t = pp.tile(shape, dtype, tag=dram.name + "_sb")
                nc.sync.dma_start(t[:], dram[:])
                return t

            c_mcum = load_const(d_mcum, [C, C])
            c_mcen = load_const(d_mcen, [C, C])
            c_mrev = load_const(d_mrev, [C, C])
            c_negones = load_const(d_negones, [C, 128])
            c_negcol = load_const(d_negcol, [C, 1])
            c_trimask = load_const(d_trimask, [C, C], BF16)
            c_identb = load_const(d_identb, [128, 128], BF16)
            c_ones1 = load_const(d_ones1, [1, C], BF16)
            c_wf2 = load_const(d_wf2, [DV, HL * M], BF16)
            c_wg2 = load_const(d_wg2, [DV, HL * DV], BF16)
            c_bg2 = load_const(d_bg2, [1, HL * DV], BF16)
            c_convw = load_const(d_convw, [128, 8, KW])
            c_eps6 = pp.tile([C, 1], F32, tag="c_eps6")
            nc.vector.memset(c_eps6[:], 1e-6)

            # ---------- big loads ----------
            xt_sb = pp.tile([128, NKT, T], BF16, tag="xt_sb")
            xtr = d_xt.rearrange("(k p) t -> k p t", p=128)
            for kt in range(NKT):
                nc.sync.dma_start(xt_sb[:, kt, :], xtr[kt])
            wbr = d_wbig.rearrange("(k p) c -> k p c", p=128)
            wb_sb = pp.tile([128, NKT, HL], BF16, tag="wb_sb")
            wbbr = d_wb.rearrange("(k p) c -> k p c", p=128)
            for kt in range(NKT):
                nc.sync.dma_start(wb_sb[:, kt, :], wbbr[kt])
            wo_sb = pp.tile([128, HL, HID], BF16, tag="wo_sb")
            wor = d_wo.rearrange("(h p) o -> h p o", p=128)
            for h in range(HL):
                nc.sync.dma_start(wo_sb[:, h, :], wor[h])

            # ---------- P1: projections + conv + silu ----------
            # conv outputs, channel-major [chan, t]; q pre-scaled by SCALE
            f1T = pp.tile([128, T], BF16, tag="f1T")
            g1T = pp.tile([128, T], BF16, tag="g1T")
            qT = pp.tile([128, HL, T], BF16, tag="qT")
            kT = pp.tile([128, HL, T], BF16, tag="kT")
            vT = pp.tile([128, HL, T], BF16, tag="vT")
            wT = pp.tile([128, HL, T], BF16, tag="wT")

            def project(ct, out_ap):
                """returns 2 psum tiles [128,512] = (X @ Wbig[:, ct])^T."""
                wct = scr.tile([128, NKT, 128], BF16, tag="wct", bufs=3)
                for kt in range(NKT):
                    nc.sync.dma_start(wct[:, kt, :],
                                      wbr[kt][:, ct * 128:(ct + 1) * 128])
                acc = []
                for tt in range(2):
                    ps = psA.tile([128, 512], F32, tag="pA")
                    for kt in range(NKT):
                        nc.tensor.matmul(
                            ps[:],
                            wct[:, kt, :],
                            xt_sb[:, kt, tt * 512:(tt + 1) * 512],
                            start=(kt == 0), stop=(kt == NKT - 1))
                    acc.append(ps)
                return acc

            def conv_silu(acc, cw_col, out_ap, scale):
                """causal conv (KW taps) + silu via tanh; acc: 2 psum tiles."""
                xpad = scr.tile([128, T + KW - 1], BF16, tag="xpad", bufs=2)
                nc.gpsimd.memset(xpad[:, 0:KW - 1], 0.0)
                for tt in range(2):
                    nc.scalar.copy(
                        xpad[:, KW - 1 + tt * 512: KW - 1 + (tt + 1) * 512],
                        acc[tt][:])
                cacc = scr.tile([128, T], BF16, tag="cacc", bufs=2)
                nc.vector.tensor_scalar_mul(
                    cacc[:], xpad[:, 0:T], c_convw[:, cw_col, 0:1])
                for i in range(1, KW):
                    nc.vector.scalar_tensor_tensor(
                        cacc[:], xpad[:, i:i + T], c_convw[:, cw_col, i:i + 1],
                        cacc[:], op0=ALU.mult, op1=ALU.add)
                th = scr.tile([128, T], BF16, tag="th", bufs=2)
                nc.scalar.activation(th[:], cacc[:], AF.Tanh, scale=0.5)
                sg = scr.tile([128, T], BF16, tag="sg", bufs=2)
                nc.gpsimd.tensor_scalar(
                    sg[:], th[:], 0.5 * scale, 0.5 * scale,
                    ALU.mult, ALU.add)
                nc.gpsimd.tensor_mul(out_ap, cacc[:], sg[:])

            # order: f1, w, g1, beta first (P2a deps), then k, v, q
            accs = project(8, None)
            for tt in range(2):
                nc.scalar.copy(f1T[:, tt * 512:(tt + 1) * 512], accs[tt][:])
            for h in range(HL):
                conv_silu(project(6 + h, None), 6 + h, wT[:, h, :], 1.0)
            accs = project(9, None)
            for tt in range(2):
                nc.scalar.copy(g1T[:, tt * 512:(tt + 1) * 512], accs[tt][:])
            # beta: [2, T] tiny
            beta_sb = pp.tile([HL, T], BF16, tag="beta_sb")
            for tt in range(2):
                ps = psA.tile([HL, 512], F32, tag="pA")
                for kt in range(NKT):
                    nc.tensor.matmul(
                        ps[:], wb_sb[:, kt, :],
                        xt_sb[:, kt, tt * 512:(tt + 1) * 512],
                        start=(kt == 0), stop=(kt == NKT - 1))
                bth = scr.tile([HL, 512], F32, tag="bth")
                nc.scalar.activation(bth[:], ps[:], AF.Tanh, scale=0.5)
                nc.vector.tensor_scalar(
                    beta_sb[:, tt * 512:(tt + 1) * 512], bth[:],
                    0.5, 0.5, ALU.mult, ALU.add)
            for h in range(HL):
                conv_silu(project(2 + h, None), 2 + h, kT[:, h, :], 1.0)
            for h in range(HL):
                conv_silu(project(4 + h, None), 4 + h, vT[:, h, :], 1.0)
            for h in range(HL):
                conv_silu(project(0 + h, None), 0 + h, qT[:, h, :], SCALE)

            # ---------- P2a: gate logits + l2norm scalars (batched Ln) ----------
            gpos_all = pp.tile([C, NCH, HL * M], F32, tag="gpos_all")
            wps_all = pp.tile([C, NCH, HL * M], BF16, tag="wps_all")
            beta_c = pp.tile([C, NCH, HL], F32, tag="beta_c")
            ss_all = pp.tile([C, NCH, HL], F32, tag="ss_all")
            rsb_all = pp.tile([C, NCH, HL], F32, tag="rsb_all")
            for n in range(NCH):
                t0 = n * C
                gps = psB.tile([C, HL * M], F32, tag="pB")
                nc.tensor.matmul(gps[:], f1T[:, t0:t0 + C], c_wf2[:],
                                 start=True, stop=True)
                nc.scalar.activation(gpos_all[:, n, :], gps[:], AF.Exp,
                                     scale=-1.0)
                # w transposes + beta transpose
                for h in range(HL):
                    tp = psC.tile([C, 128], BF16, tag="pC")
                    nc.tensor.transpose(tp[:], wT[:, h, t0:t0 + C], c_identb[:])
                    nc.scalar.copy(wps_all[:, n, h * M:(h + 1) * M], tp[:])
                bt = psC.tile([C, HL], BF16, tag="pC")
                nc.tensor.transpose(bt[:], beta_sb[:, t0:t0 + C],
                                    c_identb[0:HL, 0:HL])
                nc.scalar.copy(beta_c[:, n, :], bt[:])
            for n in range(NCH):
                for h in range(HL):
                    w2 = scr.tile([C, M], BF16, tag="w2")
                    nc.scalar.activation(
                        w2[:], wps_all[:, n, h * M:(h + 1) * M], AF.Square,
                        accum_out=ss_all[:, n, h:h + 1])
            # --- Ln region (single act-table switch) ---
            for n in range(NCH):
                nc.scalar.activation(gpos_all[:, n, :], gpos_all[:, n, :],
                                     AF.Ln, bias=1.0)
            sd_all = pp.tile([C, NCH, HL], F32, tag="sd_all")
            for n in range(NCH):
                for h in range(HL):
                    nc.scalar.activation(sd_all[:, n, h:h + 1],
                                         ss_all[:, n, h:h + 1],
                                         AF.Ln, bias=c_eps6[:])
            # --- back to exp table ---
            for n in range(NCH):
                for h in range(HL):
                    rs = scr.tile([C, 1], F32, tag="rs")
                    nc.scalar.activation(rs[:], sd_all[:, n, h:h + 1],
                                         AF.Exp, scale=-0.5)
                    nc.vector.tensor_mul(rsb_all[:, n, h:h + 1], rs[:],
                                         beta_c[:, n, h:h + 1])

            # ---------- P2b: per-chunk decay tensors + intra scores ----------
            Lam_all = pp.tile([C, NCH, HL * M], BF16, tag="Lam_all")
            Epos_all = pp.tile([C, NCH, HL * M], BF16, tag="Epos_all")
            Eneg_all = pp.tile([C, NCH, HL * M], BF16, tag="Eneg_all")
            EnegT_all = pp.tile([M, NCH, HL * C], BF16, tag="EnegT_all")
            Kdec_all = pp.tile([C, NCH, HL * M], BF16, tag="Kdec_all")
            LamCb_all = pp.tile([128, NCH, HL * M], BF16, tag="LamCb_all")
            LamCc_all = pp.tile([M, NCH, HL], F32, tag="LamCc_all")
            Kc_all = pp.tile([C, NCH, HL * DK], BF16, tag="Kc_all")
            Vc_all = pp.tile([C, NCH, HL * DV], BF16, tag="Vc_all")
            s2_all = pp.tile([C, NCH, HL * M], BF16, tag="s2_all")

            for n in range(NCH):
                t0 = n * C
                gsl = gpos_all[:, n, :]
                pG = psB.tile([C, 2, HL * M], F32, tag="pB")
                nc.tensor.matmul(pG[:, 0, :], c_mcum[:], gsl, start=True, stop=True)
                nc.tensor.matmul(pG[:, 1, :], c_mcen[:], gsl, start=True, stop=True)
                pG2 = psB.tile([C, 2, HL * M], F32, tag="pB")
                nc.tensor.matmul(pG2[:, 0, :], c_mrev[:], gsl, start=True, stop=True)
                nc.tensor.matmul(pG2[:, 1, :], c_negones[:], gsl, start=True, stop=True)
                nc.scalar.activation(Lam_all[:, n, :], pG[:, 0, :], AF.Exp)
                nc.scalar.activation(Epos_all[:, n, :], pG[:, 1, :], AF.Exp)
                enege = scr.tile([C, HL * M], BF16, tag="enege")
                nc.scalar.activation(enege[:], pG[:, 1, :], AF.Exp, scale=-1.0)
                ereve = scr.tile([C, HL * M], BF16, tag="ereve")
                nc.scalar.activation(ereve[:], pG2[:, 0, :], AF.Exp)
                nc.scalar.activation(LamCb_all[:, n, :], pG2[:, 1, :], AF.Exp)
                for h in range(HL):
                    pLcc = psC.tile([M, 1], F32, tag="pC")
                    nc.tensor.matmul(pLcc[:], gsl[:, h * M:(h + 1) * M],
                                     c_negcol[:], start=True, stop=True)
                    nc.scalar.activation(LamCc_all[:, n, h:h + 1], pLcc[:],
                                         AF.Exp)
                # bw, Eneg, Kdec
                bw = scr.tile([C, HL * M], F32, tag="bw")
                for h in range(HL):
                    nc.vector.tensor_scalar_mul(
                        bw[:, h * M:(h + 1) * M],
                        wps_all[:, n, h * M:(h + 1) * M],
                        rsb_all[:, n, h:h + 1])
                nc.vector.tensor_mul(Eneg_all[:, n, :], enege[:], bw[:])
                nc.vector.tensor_mul(Kdec_all[:, n, :], ereve[:], bw[:])
                # K/V/EnegT transposes
                for h in range(HL):
                    tp = psC.tile([C, 128], BF16, tag="pC")
                    nc.tensor.transpose(tp[:], kT[:, h, t0:t0 + C], c_identb[:])
                    nc.scalar.copy(Kc_all[:, n, h * DK:(h + 1) * DK], tp[:])
                    tp2 = psC.tile([C, 128], BF16, tag="pC")
                    nc.tensor.transpose(tp2[:], vT[:, h, t0:t0 + C], c_identb[:])
                    nc.scalar.copy(Vc_all[:, n, h * DV:(h + 1) * DV], tp2[:])
                    tp3 = psC.tile([M, C], BF16, tag="pC")
                    nc.tensor.transpose(
                        tp3[:],
                        Eneg_all[:, n, h * M:(h + 1) * M], c_identb[:])
                    nc.scalar.copy(EnegT_all[:, n, h * C:(h + 1) * C], tp3[:])
                # pt + mask + intra + s2
                for h in range(HL):
                    ppt = psC.tile([C, C], F32, tag="pC")
                    nc.tensor.matmul(ppt[:], kT[:, h, t0:t0 + C],
                                     qT[:, h, t0:t0 + C], start=True, stop=True)
                    Ptm = scr.tile([C, C], BF16, tag="Ptm")
                    nc.vector.tensor_mul(Ptm[:], ppt[:], c_trimask[:])
                    pin = psC.tile([C, M], F32, tag="pC")
                    nc.tensor.matmul(pin[:], Ptm[:],
                                     Eneg_all[:, n, h * M:(h + 1) * M],
                                     start=True, stop=True)
                    nc.vector.tensor_mul(
                        s2_all[:, n, h * M:(h + 1) * M], pin[:],
                        Epos_all[:, n, h * M:(h + 1) * M])

            # ---------- P3: sequential scan core ----------
            Sk = [pp.tile([DK, M], F32, name=f"Sk{h}", tag=f"Sk{h}") for h in range(HL)]
            Sv = [pp.tile([M, DV], F32, name=f"Sv{h}", tag=f"Sv{h}") for h in range(HL)]
            Skb = [pp.tile([DK, M], BF16, name=f"Skb{h}", tag=f"Skb{h}") for h in range(HL)]
            Svb = [pp.tile([M, DV], BF16, name=f"Svb{h}", tag=f"Svb{h}") for h in range(HL)]
            for h in range(HL):
                nc.vector.memset(Sk[h][:], 0.0)
                nc.vector.memset(Sv[h][:], 0.0)
                nc.gpsimd.memset(Skb[h][:], 0.0)
                nc.gpsimd.memset(Svb[h][:], 0.0)
            o_pre = pp.tile([C, NCH, HL * DV], F32, tag="o_pre")
            dens = pp.tile([C, NCH, HL], F32, tag="dens")

            for n in range(NCH):
                for h in range(HL):
                    hs = slice(h * M, (h + 1) * M)
                    hc = slice(h * C, (h + 1) * C)
                    t0 = n * C
                    # scores
                    pqs = psC.tile([C, M], F32, tag="pC")
                    nc.tensor.matmul(pqs[:], qT[:, h, t0:t0 + C], Skb[h][:],
                                     start=True, stop=True)
                    sS = scr.tile([C, M], F32, tag="sS")
                    nc.vector.tensor_mul(sS[:], pqs[:], Lam_all[:, n, hs])
                    nc.vector.tensor_add(sS[:], sS[:], s2_all[:, n, hs])
                    pexp = scr.tile([C, M], BF16, tag="pexp")
                    nc.scalar.activation(pexp[:], sS[:], AF.Exp,
                                         accum_out=dens[:, n, h:h + 1])
                    aL = scr.tile([C, M], BF16, tag="aL")
                    nc.gpsimd.tensor_mul(aL[:], pexp[:], Lam_all[:, n, hs])
                    aE = scr.tile([C, M], BF16, tag="aE")
                    nc.gpsimd.tensor_mul(aE[:], pexp[:], Epos_all[:, n, hs])
                    # transposes
                    paLT = psC.tile([M, C], BF16, tag="pC")
                    nc.tensor.transpose(paLT[:], aL[:], c_identb[:])
                    aLT = scr.tile([M, C], BF16, tag="aLT")
                    nc.scalar.copy(aLT[:], paLT[:])
                    paET = psC.tile([M, C], BF16, tag="pC")
                    nc.tensor.transpose(paET[:], aE[:], c_identb[:])
                    aET = scr.tile([M, C], BF16, tag="aET")
                    nc.scalar.copy(aET[:], paET[:])
                    # rt in 2 blocks ([j,i]; skip overflowing j>=HC,i<HC corner)
                    prt = psC.tile([C, C], F32, tag="pC")
                    nc.vector.memset(prt[HC:C, 0:HC], 0.0)
                    nc.tensor.matmul(prt[0:HC, 0:HC],
                                     EnegT_all[:, n, h * C:h * C + HC],
                                     aET[:, 0:HC], start=True, stop=True)
                    nc.tensor.matmul(prt[:, HC:C],
                                     EnegT_all[:, n, hc],
                                     aET[:, HC:C], start=True, stop=True)
                    Rmt = scr.tile([C, C], BF16, tag="Rmt")
                    nc.vector.tensor_mul(Rmt[:], prt[:], c_trimask[:])
                    # output
                    po = psC.tile([C, DV], F32, tag="pC")
                    nc.tensor.matmul(po[:], aLT[:], Svb[h][:],
                                     start=True, stop=False)
                    nc.tensor.matmul(po[:], Rmt[:],
                                     Vc_all[:, n, h * DV:(h + 1) * DV],
                                     start=False, stop=True)
                    nc.scalar.copy(o_pre[:, n, h * DV:(h + 1) * DV], po[:])
                    # state updates
                    pskk = psC.tile([DK, M], F32, tag="pC")
                    nc.tensor.matmul(pskk[:],
                                     Kc_all[:, n, h * DK:(h + 1) * DK],
                                     Kdec_all[:, n, hs], start=True, stop=True)
                    skt = scr.tile([DK, M], F32, tag="skt")
                    nc.vector.tensor_mul(skt[:], Sk[h][:], LamCb_all[:, n, hs])
                    nc.vector.tensor_add(Sk[h][:], skt[:], pskk[:])
                    nc.gpsimd.tensor_copy(Skb[h][:], Sk[h][:])
                    psvk = psC.tile([M, DV], F32, tag="pC")
                    nc.tensor.matmul(psvk[:], Kdec_all[:, n, hs],
                                     Vc_all[:, n, h * DV:(h + 1) * DV],
                                     start=True, stop=True)
                    svt = scr.tile([M, DV], F32, tag="svt")
                    nc.vector.tensor_scalar_mul(svt[:], Sv[h][:],
                                                LamCc_all[:, n, h:h + 1])
                    nc.vector.tensor_add(Sv[h][:], svt[:], psvk[:])
                    nc.gpsimd.tensor_copy(Svb[h][:], Sv[h][:])

            # ---------- P4: gate + RMSNorm epilogue (batched Ln) ----------
            oT = pp.tile([128, HL, T], BF16, tag="oT")
            oss = pp.tile([C, NCH, HL], F32, tag="oss")
            epsb = pp.tile([C, NCH, HL], F32, tag="epsb")
            om_all = pp.tile([C, NCH, HL], F32, tag="om_all")
            sg_list = []
            for n in range(NCH):
                t0 = n * C
                pgt = psB.tile([C, HL * DV], F32, tag="pB")
                nc.tensor.matmul(pgt[:], g1T[:, t0:t0 + C], c_wg2[:],
                                 start=True, stop=False)
                nc.tensor.matmul(pgt[:], c_ones1[:], c_bg2[:],
                                 start=False, stop=True)
                gth = scr.tile([C, HL * DV], BF16, tag="gth", bufs=8)
                nc.scalar.activation(gth[:], pgt[:], AF.Tanh, scale=0.5)
                sg = scr.tile([C, HL * DV], BF16, tag="sgate", bufs=8)
                nc.gpsimd.tensor_scalar(sg[:], gth[:], 0.5, 0.5,
                                        ALU.mult, ALU.add)
                sg_list.append(sg)
                for h in range(HL):
                    o2 = scr.tile([C, DV], BF16, tag="o2")
                    nc.scalar.activation(
                        o2[:], o_pre[:, n, h * DV:(h + 1) * DV], AF.Square,
                        accum_out=oss[:, n, h:h + 1])
                    nc.vector.scalar_tensor_tensor(
                        epsb[:, n, h:h + 1], dens[:, n, h:h + 1], EPS,
                        dens[:, n, h:h + 1], op0=ALU.mult, op1=ALU.mult)
            # Ln region
            for n in range(NCH):
                for h in range(HL):
                    nc.scalar.activation(om_all[:, n, h:h + 1],
                                         oss[:, n, h:h + 1], AF.Ln,
                                         scale=1.0 / DV,
                                         bias=epsb[:, n, h:h + 1])
            # back to exp table; finish + transpose
            for n in range(NCH):
                t0 = n * C
                for h in range(HL):
                    rmsf = scr.tile([C, 1], F32, tag="rmsf")
                    nc.scalar.activation(rmsf[:], om_all[:, n, h:h + 1],
                                         AF.Exp, scale=-0.5)
                    of = scr.tile([C, DV], BF16, tag="of")
                    nc.vector.scalar_tensor_tensor(
                        of[:], o_pre[:, n, h * DV:(h + 1) * DV], rmsf[:],
                        sg_list[n][:, h * DV:(h + 1) * DV],
                        op0=ALU.mult, op1=ALU.mult)
                    pot = psC.tile([DV, C], BF16, tag="pC")
                    nc.tensor.transpose(pot[:], of[:], c_identb[:])
                    nc.scalar.copy(oT[:, h, t0:t0 + C], pot[:])

            # ---------- P5: output projection ----------
            for tt in range(8):
                for cl in range(4):
                    ps = psA.tile([128, 512], F32, tag="pA")
                    for h in range(HL):
                        nc.tensor.matmul(
                            ps[:],
                            oT[:, h, tt * 128:(tt + 1) * 128],
                            wo_sb[:, h, cl * 512:(cl + 1) * 512],
                            start=(h == 0), stop=(h == HL - 1))
                    osb = scr.tile([128, 512], FP16, tag="outsb", bufs=3)
                    nc.scalar.copy(osb[:], ps[:])
                    nc.sync.dma_start(
                        d_out[tt * 128:(tt + 1) * 128, cl * 512:(cl + 1) * 512],
                        osb[:])
    nc.compile()
    return nc


def _host_inputs(inputs):
    f32 = np.float32
    bf16 = ml_dtypes.bfloat16
    X = np.ascontiguousarray(np.asarray(inputs["hidden_states"], f32)[0])
    XT = np.ascontiguousarray(X.T).astype(bf16)

    j = np.arange(C)[:, None]
    i = np.arange(C)[None, :]
    mcum = -((j <= i).astype(f32))
    mcen = -((j <= i).astype(f32)) + (j <= HC - 1).astype(f32)
    mrev = -((j > i).astype(f32))
    negones = np.full((C, 128), -1.0, f32)
    negcol = np.full((C, 1), -1.0, f32)
    trimask = (j <= i).astype(f32).astype(bf16)
    identb = np.eye(128, dtype=f32).astype(bf16)
    ones1 = np.ones((1, C), f32).astype(bf16)

    Wo_full = np.asarray(inputs["Wo"], f32) * np.tile(
        np.asarray(inputs["norm_w"], f32), H)[:, None]

    Wq = np.asarray(inputs["Wq"], f32)
    Wk = np.asarray(inputs["Wk"], f32)
    Wv = np.asarray(inputs["Wv"], f32)
    Ww = np.asarray(inputs["Ww"], f32)
    Wf1 = np.asarray(inputs["Wf1"], f32)
    Wg1 = np.asarray(inputs["Wg1"], f32)
    cq = np.asarray(inputs["cq"], f32)
    ck = np.asarray(inputs["ck"], f32)
    cv = np.asarray(inputs["cv"], f32)

    in_maps = []
    for c in range(8):
        hsl = slice(c * HL * 128, (c + 1) * HL * 128)
        bsl = slice(c * HL, (c + 1) * HL)
        wbig = np.concatenate(
            [Wq[:, hsl], Wk[:, hsl], Wv[:, hsl], Ww[:, hsl], Wf1, Wg1],
            axis=1)
        # conv weights per ct: order q0 q1 k0 k1 v0 v1 w0 w1
        convw = np.zeros((128, 8, KW), f32)
        for hh in range(HL):
            ch = slice((c * HL + hh) * 128, (c * HL + hh + 1) * 128)
            convw[:, 0 + hh] = cq[ch]
            convw[:, 2 + hh] = ck[ch]
            convw[:, 4 + hh] = cv[ch]
            convw[:, 6 + hh] = cv[ch]   # w uses v's conv (faithful to ref)
        m = {
            "xt": XT,
            "wbig": np.ascontiguousarray(wbig).astype(bf16),
            "wb": np.ascontiguousarray(
                np.asarray(inputs["Wb"], f32)[:, bsl]).astype(bf16),
            "wf2": np.ascontiguousarray(
                np.asarray(inputs["Wf2"], f32)[:, hsl]).astype(bf16),
            "wg2": np.ascontiguousarray(
                np.asarray(inputs["Wg2"], f32)[:, hsl]).astype(bf16),
            "bg2": np.ascontiguousarray(
                np.asarray(inputs["bg2"], f32)[None, hsl]).astype(bf16),
            "wo": np.ascontiguousarray(Wo_full[hsl]).astype(bf16),
            "convw": convw,
            "mcum": mcum, "mcen": mcen, "mrev": mrev,
            "negones": negones, "negcol": negcol,
            "trimask": trimask, "identb": identb, "ones1": ones1,
        }
        in_maps.append(m)
    return in_maps


def kernel(_trace=False, **inputs):
    if "nc" not in _CACHE:
        _CACHE["nc"] = _build_nc()
    nc = _CACHE["nc"]
    in_maps = _host_inputs(inputs)
    res = run_bass_kernel_spmd(nc, in_maps, core_ids=list(range(8)),
                               trace=_trace)
    _CACHE["last_result"] = res
    out = np.zeros((T, HID), np.float32)
    for r in res.results:
        out += np.asarray(r["out"], np.float32)
    return out.reshape(B, T, HID)
